# revision 4
# baseline (speedup 1.0000x reference)
"""EntropyDispatchedLinear (int8-weight GEMM with per-column dequant) on 8 TRN2 cores.

out[m, n] = (sum_k x[m, k] * w_int8[k, n]) * w_scale[n],  x fp16 [32, 8192],
w_int8 int8 [8192, 28672], out fp16 [32, 28672].

Strategy (tensor-parallel over out_features N, 3584 columns per core), built on
three hardware facts measured on this part:
- DVE tensor_scalar uint16 bitVec ops (shift/and) run at 4x mode (~4.7
  words/cycle), twice the 2x-mode rate of int8->bf16 cast copies.  So weights
  stream as PAIRS of biased bytes packed in uint16 words; two DVE passes
  (v>>8, v&255) emit the bytes as uint16 bit patterns that ARE fp16 denormals
  (value w_u * 2^-24).  The PE multiplies fp16 denormals exactly; the 2^24
  rescale folds into the per-column scales and the +128 bias folds into a
  per-rep PSUM pre-init matmul (c2[m] x mask-row 2^-24, one per psum bank).
- Conversion engine split per strip: DVE handles 5 of 7 n-tiles (2560 cols)
  via packed passes; ACT converts 640 cols and GPSIMD 384 cols of plain signed
  int8->bf16.  Column order is permuted host-side (scales follow; kernel()
  un-permutes the gathered output in numpy - free off-device).
- The faster conversion frees SBUF streaming buffers, so residency grows:
  1 strip parked converted (fp16), 8 strips parked raw (1B/weight), 7 strips
  streamed per rep (~41 us DMA at ~322 GB/s/core vs 70 us baseline).

Matmuls: stationary = x^T k-tile [128, 32] fp16, moving = converted weight
tile [128, 512]; 7 n-tiles packed into 2 PSUM banks at column offsets
0/32/64/96 (tile_position packing).  Epilogue: psum * scale' -> fp16 out;
out-DMAs ride the scalar HWDGE ring.
"""
import numpy as np

M, K, NFULL = 32, 8192, 28672
NCORES = 8
NS = NFULL // NCORES          # 3584 columns per core
KT = K // 128                 # 64 k-tiles
STRIP_KT = 4                  # k-tiles per strip
NSTRIP = KT // STRIP_KT       # 16
NT = NS // 512                # 7 n-tiles

# conversion shares (columns per strip-row); D = DVE packed, A = ACT, G = GPSIMD
D_PACK = 2560                 # 5 n-tiles, packed as D_PACK//2 uint16 words
A_ACT = 640
G_GP = NS - D_PACK - A_ACT    # 384
DW = D_PACK // 2              # 1280 packed words per row
NPLAIN = NS - D_PACK          # 1024 plain cols per row

S_STREAM = list(range(7))     # streamed strips
S_RES_RAW = list(range(7, 15))   # resident raw (packed/plain byte) strips
S_RES_CONV = [15]             # resident pre-converted strips
# processing order: converted-resident first (PE work before first DMA lands),
# raw-resident spread between streamed strips to fill conversion-engine gaps
ORDER = [15, 7, 0, 8, 1, 9, 2, 10, 3, 11, 4, 12, 5, 13, 6, 14]

_CACHE = {}


def _build(reps=1):
    import concourse.bacc as bacc
    import concourse.mybir as mybir
    import concourse.tile as tile

    nc = bacc.Bacc("TRN2", target_bir_lowering=False, debug=False, num_devices=NCORES)
    dt = mybir.dt
    op = mybir.AluOpType
    xT = nc.dram_tensor("xT", [K, M], dt.float16, kind="ExternalInput").ap()
    wp = nc.dram_tensor("wp", [NSTRIP, 128, STRIP_KT * DW], dt.uint16,
                        kind="ExternalInput").ap()
    wq = nc.dram_tensor("wq", [NSTRIP, 128, STRIP_KT * NPLAIN], dt.int8,
                        kind="ExternalInput").ap()
    scaleA = nc.dram_tensor("scaleA", [128, 512], dt.float32, kind="ExternalInput").ap()
    scaleB = nc.dram_tensor("scaleB", [128, 512], dt.float32, kind="ExternalInput").ap()
    c2A = nc.dram_tensor("c2A", [1, 128], dt.float16, kind="ExternalInput").ap()
    c2B = nc.dram_tensor("c2B", [1, 128], dt.float16, kind="ExternalInput").ap()
    out = nc.dram_tensor("out", [M, NS], dt.float16, kind="ExternalOutput").ap()

    xT_t = xT.rearrange("(kt p) m -> p kt m", p=128)
    H = STRIP_KT // 2  # rows per half strip

    with tile.TileContext(nc) as tc:
        with (
            tc.tile_pool(name="xp", bufs=1) as xp,
            tc.tile_pool(name="wraw", bufs=2) as wrawp,
            tc.tile_pool(name="wbf", bufs=2) as wbfp,
            tc.tile_pool(name="op", bufs=2) as outp,
            tc.tile_pool(name="ps", bufs=2, space="PSUM") as psp,
        ):
            # ---- preloads (outside the steady-state loop) ----
            xsb = xp.tile([128, KT, M], dt.float16, tag="x")
            nc.gpsimd.dma_start(xsb[:], xT_t)
            scA = xp.tile([128, 512], dt.float32, tag="scA")
            nc.gpsimd.dma_start(scA[:], scaleA)
            scB = xp.tile([128, 512], dt.float32, tag="scB")
            nc.gpsimd.dma_start(scB[:], scaleB)
            c2a = xp.tile([1, 128], dt.float16, tag="c2a")
            nc.gpsimd.dma_start(c2a[:], c2A)
            c2b = xp.tile([1, 128], dt.float16, tag="c2b")
            nc.gpsimd.dma_start(c2b[:], c2B)
            mrow = xp.tile([1, 512], dt.float16, tag="mrow")
            nc.vector.memset(mrow[:], 2.0 ** -24)

            res_raw = {}
            for s in S_RES_RAW:
                tp = xp.tile([128, STRIP_KT, DW], dt.uint16, tag=f"rp_{s}")
                nc.sync.dma_start(tp[:], wp[s].rearrange("p (t w) -> p t w", t=STRIP_KT))
                tq = xp.tile([128, STRIP_KT, NPLAIN], dt.int8, tag=f"rq_{s}")
                nc.sync.dma_start(tq[:], wq[s].rearrange("p (t w) -> p t w", t=STRIP_KT))
                res_raw[s] = (tp, tq)

            def convert(dst, srcp, srcq):
                """Convert a half-strip: srcp [128,H,DW] uint16 packed pairs and
                srcq [128,H,NPLAIN] int8 into dst fp16 [128,H,NS]."""
                du = dst.bitcast(dt.uint16)
                # evens: low bytes -> cols [0, DW)
                nc.vector.tensor_scalar(
                    du[:, :, 0:DW], srcp[:], 255, None, op.bitwise_and)
                # odds: high bytes -> cols [DW, 2*DW)
                nc.vector.tensor_scalar(
                    du[:, :, DW:D_PACK], srcp[:], 8, None, op.logical_shift_right)
                db = dst.bitcast(dt.bfloat16)
                nc.scalar.copy(db[:, :, D_PACK:D_PACK + A_ACT], srcq[:, :, 0:A_ACT])
                if G_GP:
                    nc.gpsimd.tensor_copy(db[:, :, D_PACK + A_ACT:NS],
                                          srcq[:, :, A_ACT:NPLAIN])

            def matmuls(pA, pB, s, wtile, t0):
                """Matmuls for rows [t0, t0+wtile.shape[1]) of strip s; wtile
                indexed by local row."""
                wb = wtile.bitcast(dt.bfloat16)
                for lt in range(H):
                    t = t0 + lt
                    kt = s * STRIP_KT + t
                    for nt_i in range(NT):
                        p, j = (pA, nt_i) if nt_i < 4 else (pB, nt_i - 4)
                        mv = wtile if nt_i < 5 else wb
                        nc.tensor.matmul(
                            p[32 * j:32 * j + 32, :],
                            xsb[:, kt, :],
                            mv[:, lt, 512 * nt_i:512 * (nt_i + 1)],
                            start=False,
                            stop=(s == ORDER[-1] and t == STRIP_KT - 1),
                            tile_position=(0, 32 * j),
                            skip_group_check=True,
                        )

            res_conv = {}
            for s in S_RES_CONV:
                tcv = xp.tile([128, STRIP_KT, NS], dt.float16, tag=f"rc_{s}")
                wpv = wp[s].rearrange("p (t w) -> p t w", t=STRIP_KT)
                wqv = wq[s].rearrange("p (t w) -> p t w", t=STRIP_KT)
                for h in range(STRIP_KT // H):
                    t0, t1 = h * H, (h + 1) * H
                    tmpp = wrawp.tile([128, H, DW], dt.uint16, tag="wps")
                    nc.sync.dma_start(tmpp[:], wpv[:, t0:t1, :])
                    tmpq = wrawp.tile([128, H, NPLAIN], dt.int8, tag="wqs")
                    nc.sync.dma_start(tmpq[:], wqv[:, t0:t1, :])
                    convert(tcv[:, t0:t1, :], tmpp, tmpq)
                res_conv[s] = tcv

            def half_tiles(s):
                """Yield (t0, converted fp16 half tile) for strip s."""
                if s in S_RES_RAW:
                    srcp, srcq = res_raw[s]
                    for h in range(STRIP_KT // H):
                        t0, t1 = h * H, (h + 1) * H
                        wcv = wbfp.tile([128, H, NS], dt.float16, tag="wcv")
                        convert(wcv, srcp[:, t0:t1, :], srcq[:, t0:t1, :])
                        yield t0, wcv
                else:
                    wpv = wp[s].rearrange("p (t w) -> p t w", t=STRIP_KT)
                    wqv = wq[s].rearrange("p (t w) -> p t w", t=STRIP_KT)
                    for h in range(STRIP_KT // H):
                        t0, t1 = h * H, (h + 1) * H
                        srcp = wrawp.tile([128, H, DW], dt.uint16, tag="wps")
                        srcq = wrawp.tile([128, H, NPLAIN], dt.int8, tag="wqs")
                        nc.sync.dma_start(srcp[:], wpv[:, t0:t1, :])
                        nc.sync.dma_start(srcq[:], wqv[:, t0:t1, :])
                        wcv = wbfp.tile([128, H, NS], dt.float16, tag="wcv")
                        convert(wcv, srcp, srcq)
                        yield t0, wcv

            def body():
                pA = psp.tile([128, 512], dt.float32, tag="pA")
                pB = psp.tile([128, 512], dt.float32, tag="pB")
                # psum pre-init: pX[32j+m, f] = c2X[32j+m] * 2^-24
                nc.tensor.matmul(pA[:], c2a[:], mrow[:], start=True, stop=False,
                                 skip_group_check=True)
                nc.tensor.matmul(pB[:], c2b[:], mrow[:], start=True, stop=False,
                                 skip_group_check=True)
                for s in ORDER:
                    if s in S_RES_CONV:
                        for h in range(STRIP_KT // H):
                            matmuls(pA, pB, s, res_conv[s][:, h * H:(h + 1) * H, :],
                                    h * H)
                        continue
                    for t0, wcv in half_tiles(s):
                        matmuls(pA, pB, s, wcv, t0)

                oA = outp.tile([128, 512], dt.float16, tag="oA")
                nc.vector.tensor_mul(oA[:], pA[:], scA[:])
                oB = outp.tile([96, 512], dt.float16, tag="oB")
                nc.vector.tensor_mul(oB[:], pB[0:96, :], scB[0:96, :])
                outA_view = out[:, 0:2048].rearrange("m (j f) -> j m f", f=512)
                nc.scalar.dma_start(outA_view, oA[:])
                outB_view = out[:, 2048:NS].rearrange("m (j f) -> j m f", f=512)
                nc.scalar.dma_start(outB_view, oB[:])

            if reps == 1:
                body()
            else:
                with tc.For_i(0, reps, 1, staggered_reset=True,
                              hint_engines=(mybir.EngineType.PE,)):
                    body()
    nc.compile()
    return nc


def get_nc(reps=1):
    if reps not in _CACHE:
        _CACHE[reps] = _build(reps)
    return _CACHE[reps]


def _col_perm():
    """new_col -> orig_col mapping within a core's NS columns."""
    perm = np.empty(NS, np.int64)
    perm[0:DW] = 2 * np.arange(DW)                    # evens of packed region
    perm[DW:D_PACK] = 2 * np.arange(DW) + 1           # odds of packed region
    perm[D_PACK:NS] = np.arange(D_PACK, NS)           # plain region unchanged
    return perm


_PERM = _col_perm()


def shard_inputs(x, w_int8, w_scale):
    """Full inputs -> list of 8 per-core input dicts (host-side pack/permute)."""
    x = np.asarray(x)
    if x.dtype != np.float16:
        x = x.astype(np.float16)
    w_int8 = np.asarray(w_int8)
    if w_int8.dtype != np.int8:
        w_int8 = w_int8.astype(np.int8)
    w_scale = np.asarray(w_scale)
    if w_scale.dtype != np.float32:
        w_scale = w_scale.astype(np.float32)
    x2d = x.reshape(-1, K)
    assert x2d.shape == (M, K), f"unexpected x shape {x.shape}"
    xT = np.ascontiguousarray(x2d.T)
    sumx = x2d.astype(np.float32).sum(axis=1)         # [M]
    c = (-128.0 * sumx).astype(np.float16)            # [M]
    c2A = np.tile(c, 4)[None, :]                      # [1, 128]
    c2B = np.concatenate([c, np.zeros(96, np.float16)])[None, :]

    in_maps = []
    for ci in range(NCORES):
        ws = w_scale[ci * NS:(ci + 1) * NS][_PERM]    # permuted scales
        fac = np.ones(NS, np.float64)
        fac[0:D_PACK] = 2.0 ** 24                     # packed cols: psum is 2^-24-scaled
        wsp = (ws.astype(np.float64) * fac).astype(np.float32)
        scA = np.empty((128, 512), np.float32)
        scB = np.zeros((128, 512), np.float32)
        for j in range(4):
            scA[32 * j:32 * j + 32, :] = wsp[512 * j:512 * (j + 1)][None, :]
        for j in range(3):
            scB[32 * j:32 * j + 32, :] = wsp[2048 + 512 * j:2048 + 512 * (j + 1)][None, :]

        wc = w_int8[:, ci * NS:(ci + 1) * NS]         # [K, NS]
        # [s, t, p, col] with k = (4s+t)*128 + p
        w4 = wc.reshape(NSTRIP, STRIP_KT, 128, NS)
        # packed region: biased bytes, pairs (2j, 2j+1) -> uint16 word j
        wu = (w4[:, :, :, 0:D_PACK].astype(np.int16) + 128).astype(np.uint8)
        wp_ = (wu[:, :, :, 0::2].astype(np.uint16)
               | (wu[:, :, :, 1::2].astype(np.uint16) << 8))
        wp_ = np.ascontiguousarray(
            wp_.transpose(0, 2, 1, 3).reshape(NSTRIP, 128, STRIP_KT * DW))
        # plain region: raw signed bytes
        wq_ = np.ascontiguousarray(
            w4[:, :, :, D_PACK:NS].transpose(0, 2, 1, 3).reshape(
                NSTRIP, 128, STRIP_KT * NPLAIN))
        in_maps.append({
            "xT": xT,
            "wp": wp_,
            "wq": wq_,
            "scaleA": scA,
            "scaleB": scB,
            "c2A": c2A,
            "c2B": c2B,
        })
    return in_maps


def kernel(x, w_int8, w_scale):
    """Full unsharded inputs -> full [32, 28672] fp16 output (8-core TRN2)."""
    from concourse.bass_utils import run_bass_kernel_spmd

    orig_shape = np.asarray(x).shape[:-1] + (NFULL,)
    nc = get_nc(reps=1)
    in_maps = shard_inputs(x, w_int8, w_scale)
    res = run_bass_kernel_spmd(nc, in_maps, core_ids=list(range(NCORES))).results
    inv = np.empty(NS, np.int64)
    inv[_PERM] = np.arange(NS)                        # orig_col -> new_col
    out = np.concatenate([res[c]["out"][:, inv] for c in range(NCORES)], axis=1)
    return out.reshape(orig_shape)


# revision 6
# speedup vs baseline: 1.9621x; 1.9621x over previous
"""EntropyDispatchedLinear (int8-weight GEMM with per-column dequant) on 8 TRN2 cores.

out[m, n] = (sum_k x[m, k] * w_int8[k, n]) * w_scale[n],  x fp16 [32, 8192],
w_int8 int8 [8192, 28672], out fp16 [32, 28672].

Strategy (tensor-parallel over out_features N, 3584 columns per core), built on
three hardware facts measured on this part:
- DVE tensor_scalar uint16 bitVec ops (shift/and) run at 4x mode (~4.7
  words/cycle), twice the 2x-mode rate of int8->bf16 cast copies.  So weights
  stream as PAIRS of biased bytes packed in uint16 words; two DVE passes
  (v>>8, v&255) emit the bytes as uint16 bit patterns that ARE fp16 denormals
  (value w_u * 2^-24).  The PE multiplies fp16 denormals exactly; the 2^24
  rescale folds into the per-column scales and the +128 bias folds into a
  per-rep PSUM pre-init matmul (c2[m] x mask-row 2^-24, one per psum bank).
- Conversion engine split per strip: DVE handles 5 of 7 n-tiles (2560 cols)
  via packed passes; ACT converts 640 cols and GPSIMD 384 cols of plain signed
  int8->bf16.  Column order is permuted host-side (scales follow; kernel()
  un-permutes the gathered output in numpy - free off-device).
- The faster conversion frees SBUF streaming buffers, so residency grows:
  1 strip parked converted (fp16), 8 strips parked raw (1B/weight), 7 strips
  streamed per rep (~41 us DMA at ~322 GB/s/core vs 70 us baseline).

Matmuls: stationary = x^T k-tile [128, 32] fp16, moving = converted weight
tile [128, 512]; 7 n-tiles packed into 2 PSUM banks at column offsets
0/32/64/96 (tile_position packing).  Epilogue: psum * scale' -> fp16 out;
out-DMAs ride the scalar HWDGE ring.
"""
import numpy as np

M, K, NFULL = 32, 8192, 28672
NCORES = 8
NS = NFULL // NCORES          # 3584 columns per core
KT = K // 128                 # 64 k-tiles
STRIP_KT = 4                  # k-tiles per strip
NSTRIP = KT // STRIP_KT       # 16
NT = NS // 512                # 7 n-tiles

# conversion shares (columns per strip-row); D = DVE packed, A = ACT, G = GPSIMD
D_PACK = 3072                 # 6 n-tiles, packed as D_PACK//2 uint16 words
A_ACT = 512
G_GP = NS - D_PACK - A_ACT    # 0 (gpsimd per-op dispatch overhead too high)
DW = D_PACK // 2              # 1536 packed words per row
NPLAIN = NS - D_PACK          # 512 plain cols per row
WROW = DW + NPLAIN // 2       # 1792 uint16 words per strip-row (merged buffer)

S_STREAM = list(range(7))     # streamed strips
S_RES_RAW = list(range(7, 15))   # resident raw (packed/plain byte) strips
S_RES_CONV = [15]             # resident pre-converted strips
# processing order: converted-resident first (PE work before first DMA lands),
# raw-resident spread between streamed strips to fill conversion-engine gaps
ORDER = [15, 7, 0, 8, 1, 9, 2, 10, 3, 11, 4, 12, 5, 13, 6, 14]

_CACHE = {}


def _build(reps=1):
    import concourse.bacc as bacc
    import concourse.mybir as mybir
    import concourse.tile as tile

    nc = bacc.Bacc("TRN2", target_bir_lowering=False, debug=False, num_devices=NCORES)
    dt = mybir.dt
    op = mybir.AluOpType
    xT = nc.dram_tensor("xT", [K, M], dt.float16, kind="ExternalInput").ap()
    wm = nc.dram_tensor("wm", [NSTRIP, 128, STRIP_KT * WROW], dt.uint16,
                        kind="ExternalInput").ap()
    scaleA = nc.dram_tensor("scaleA", [128, 512], dt.float32, kind="ExternalInput").ap()
    scaleB = nc.dram_tensor("scaleB", [128, 512], dt.float32, kind="ExternalInput").ap()
    c2A = nc.dram_tensor("c2A", [1, 128], dt.float16, kind="ExternalInput").ap()
    c2B = nc.dram_tensor("c2B", [1, 128], dt.float16, kind="ExternalInput").ap()
    out = nc.dram_tensor("out", [M, NS], dt.float16, kind="ExternalOutput").ap()

    xT_t = xT.rearrange("(kt p) m -> p kt m", p=128)
    H = STRIP_KT // 2  # rows per half strip

    with tile.TileContext(nc) as tc:
        with (
            tc.tile_pool(name="xp", bufs=1) as xp,
            tc.tile_pool(name="wraw", bufs=2) as wrawp,
            tc.tile_pool(name="wbf", bufs=2) as wbfp,
            tc.tile_pool(name="op", bufs=2) as outp,
            tc.tile_pool(name="ps", bufs=2, space="PSUM") as psp,
        ):
            # ---- preloads (outside the steady-state loop) ----
            xsb = xp.tile([128, KT, M], dt.float16, tag="x")
            nc.gpsimd.dma_start(xsb[:], xT_t)
            scA = xp.tile([128, 512], dt.float32, tag="scA")
            nc.gpsimd.dma_start(scA[:], scaleA)
            scB = xp.tile([128, 512], dt.float32, tag="scB")
            nc.gpsimd.dma_start(scB[:], scaleB)
            c2a = xp.tile([1, 128], dt.float16, tag="c2a")
            nc.gpsimd.dma_start(c2a[:], c2A)
            c2b = xp.tile([1, 128], dt.float16, tag="c2b")
            nc.gpsimd.dma_start(c2b[:], c2B)
            mrow = xp.tile([1, 512], dt.float16, tag="mrow")
            nc.vector.memset(mrow[:], 2.0 ** -24)

            res_raw = {}
            for s in S_RES_RAW:
                tp = xp.tile([128, STRIP_KT, WROW], dt.uint16, tag=f"rp_{s}")
                nc.sync.dma_start(tp[:], wm[s].rearrange("p (t w) -> p t w", t=STRIP_KT))
                res_raw[s] = tp

            def convert(dst, srcm):
                """Convert a half-strip: srcm [128,H,WROW] uint16 (packed pairs
                then plain bytes) into dst fp16 [128,H,NS]."""
                du = dst.bitcast(dt.uint16)
                srcp = srcm[:, :, 0:DW]
                # evens: low bytes -> cols [0, DW)
                nc.vector.tensor_scalar(
                    du[:, :, 0:DW], srcp, 255, None, op.bitwise_and)
                # odds: high bytes -> cols [DW, 2*DW)
                nc.vector.tensor_scalar(
                    du[:, :, DW:D_PACK], srcp, 8, None, op.logical_shift_right)
                db = dst.bitcast(dt.bfloat16)
                srcq = srcm.bitcast(dt.int8)  # [128, H, 2*WROW]
                nc.scalar.copy(db[:, :, D_PACK:NS],
                               srcq[:, :, 2 * DW:2 * WROW])

            def matmuls(pA, pB, s, wtile, t0):
                """Matmuls for rows [t0, t0+wtile.shape[1]) of strip s; wtile
                indexed by local row."""
                wb = wtile.bitcast(dt.bfloat16)
                for lt in range(H):
                    t = t0 + lt
                    kt = s * STRIP_KT + t
                    for nt_i in range(NT):
                        p, j = (pA, nt_i) if nt_i < 4 else (pB, nt_i - 4)
                        mv = wtile if nt_i < 6 else wb
                        nc.tensor.matmul(
                            p[32 * j:32 * j + 32, :],
                            xsb[:, kt, :],
                            mv[:, lt, 512 * nt_i:512 * (nt_i + 1)],
                            start=False,
                            stop=(s == ORDER[-1] and t == STRIP_KT - 1),
                            tile_position=(0, 32 * j),
                            skip_group_check=True,
                        )

            res_conv = {}
            for s in S_RES_CONV:
                tcv = xp.tile([128, STRIP_KT, NS], dt.float16, tag=f"rc_{s}")
                wmv = wm[s].rearrange("p (t w) -> p t w", t=STRIP_KT)
                for h in range(STRIP_KT // H):
                    t0, t1 = h * H, (h + 1) * H
                    tmpp = wrawp.tile([128, H, WROW], dt.uint16, tag="wps")
                    nc.sync.dma_start(tmpp[:], wmv[:, t0:t1, :])
                    convert(tcv[:, t0:t1, :], tmpp)
                res_conv[s] = tcv

            def half_tiles(s):
                """Yield (t0, converted fp16 half tile) for strip s."""
                if s in S_RES_RAW:
                    srcm = res_raw[s]
                    for h in range(STRIP_KT // H):
                        t0, t1 = h * H, (h + 1) * H
                        wcv = wbfp.tile([128, H, NS], dt.float16, tag="wcv")
                        convert(wcv, srcm[:, t0:t1, :])
                        yield t0, wcv
                else:
                    wmv = wm[s].rearrange("p (t w) -> p t w", t=STRIP_KT)
                    for h in range(STRIP_KT // H):
                        t0, t1 = h * H, (h + 1) * H
                        srcm = wrawp.tile([128, H, WROW], dt.uint16, tag="wps")
                        nc.sync.dma_start(srcm[:], wmv[:, t0:t1, :])
                        wcv = wbfp.tile([128, H, NS], dt.float16, tag="wcv")
                        convert(wcv, srcm)
                        yield t0, wcv

            def body():
                pA = psp.tile([128, 512], dt.float32, tag="pA")
                pB = psp.tile([128, 512], dt.float32, tag="pB")
                # psum pre-init: pX[32j+m, f] = c2X[32j+m] * 2^-24
                nc.tensor.matmul(pA[:], c2a[:], mrow[:], start=True, stop=False,
                                 skip_group_check=True)
                nc.tensor.matmul(pB[:], c2b[:], mrow[:], start=True, stop=False,
                                 skip_group_check=True)
                for s in ORDER:
                    if s in S_RES_CONV:
                        for h in range(STRIP_KT // H):
                            matmuls(pA, pB, s, res_conv[s][:, h * H:(h + 1) * H, :],
                                    h * H)
                        continue
                    for t0, wcv in half_tiles(s):
                        matmuls(pA, pB, s, wcv, t0)

                oA = outp.tile([128, 512], dt.float16, tag="oA")
                nc.vector.tensor_mul(oA[:], pA[:], scA[:])
                oB = outp.tile([96, 512], dt.float16, tag="oB")
                nc.vector.tensor_mul(oB[:], pB[0:96, :], scB[0:96, :])
                outA_view = out[:, 0:2048].rearrange("m (j f) -> j m f", f=512)
                nc.scalar.dma_start(outA_view, oA[:])
                outB_view = out[:, 2048:NS].rearrange("m (j f) -> j m f", f=512)
                nc.scalar.dma_start(outB_view, oB[:])

            if reps == 1:
                body()
            else:
                with tc.For_i(0, reps, 1, staggered_reset=True,
                              hint_engines=(mybir.EngineType.PE,)):
                    body()
    nc.compile()
    return nc


def get_nc(reps=1):
    if reps not in _CACHE:
        _CACHE[reps] = _build(reps)
    return _CACHE[reps]


def _col_perm():
    """new_col -> orig_col mapping within a core's NS columns."""
    perm = np.empty(NS, np.int64)
    perm[0:DW] = 2 * np.arange(DW)                    # evens of packed region
    perm[DW:D_PACK] = 2 * np.arange(DW) + 1           # odds of packed region
    perm[D_PACK:NS] = np.arange(D_PACK, NS)           # plain region unchanged
    return perm


_PERM = _col_perm()


def shard_inputs(x, w_int8, w_scale):
    """Full inputs -> list of 8 per-core input dicts (host-side pack/permute)."""
    x = np.asarray(x)
    if x.dtype != np.float16:
        x = x.astype(np.float16)
    w_int8 = np.asarray(w_int8)
    if w_int8.dtype != np.int8:
        w_int8 = w_int8.astype(np.int8)
    w_scale = np.asarray(w_scale)
    if w_scale.dtype != np.float32:
        w_scale = w_scale.astype(np.float32)
    x2d = x.reshape(-1, K)
    assert x2d.shape == (M, K), f"unexpected x shape {x.shape}"
    xT = np.ascontiguousarray(x2d.T)
    sumx = x2d.astype(np.float32).sum(axis=1)         # [M]
    c = (-128.0 * sumx).astype(np.float16)            # [M]
    c2A = np.tile(c, 4)[None, :]                      # [1, 128]
    npk = D_PACK // 512 - 4                           # packed tiles in bank B
    c2B = np.concatenate([np.tile(c, npk),
                          np.zeros(128 - 32 * npk, np.float16)])[None, :]

    in_maps = []
    for ci in range(NCORES):
        ws = w_scale[ci * NS:(ci + 1) * NS][_PERM]    # permuted scales
        fac = np.ones(NS, np.float64)
        fac[0:D_PACK] = 2.0 ** 24                     # packed cols: psum is 2^-24-scaled
        wsp = (ws.astype(np.float64) * fac).astype(np.float32)
        scA = np.empty((128, 512), np.float32)
        scB = np.zeros((128, 512), np.float32)
        for j in range(4):
            scA[32 * j:32 * j + 32, :] = wsp[512 * j:512 * (j + 1)][None, :]
        for j in range(3):
            scB[32 * j:32 * j + 32, :] = wsp[2048 + 512 * j:2048 + 512 * (j + 1)][None, :]

        wc = w_int8[:, ci * NS:(ci + 1) * NS]         # [K, NS]
        # [s, t, p, col] with k = (4s+t)*128 + p
        w4 = wc.reshape(NSTRIP, STRIP_KT, 128, NS)
        # packed region: biased bytes, pairs (2j, 2j+1) -> uint16 word j
        wu = (w4[:, :, :, 0:D_PACK].astype(np.int16) + 128).astype(np.uint8)
        wp_ = (wu[:, :, :, 0::2].astype(np.uint16)
               | (wu[:, :, :, 1::2].astype(np.uint16) << 8))
        # plain region: raw signed bytes, riding as uint16 word pairs
        wq_ = np.ascontiguousarray(w4[:, :, :, D_PACK:NS]).view(np.uint16)
        wm_ = np.ascontiguousarray(np.concatenate([wp_, wq_], axis=3).transpose(
            0, 2, 1, 3).reshape(NSTRIP, 128, STRIP_KT * WROW))
        in_maps.append({
            "xT": xT,
            "wm": wm_,
            "scaleA": scA,
            "scaleB": scB,
            "c2A": c2A,
            "c2B": c2B,
        })
    return in_maps


def kernel(x, w_int8, w_scale):
    """Full unsharded inputs -> full [32, 28672] fp16 output (8-core TRN2)."""
    from concourse.bass_utils import run_bass_kernel_spmd

    orig_shape = np.asarray(x).shape[:-1] + (NFULL,)
    nc = get_nc(reps=1)
    in_maps = shard_inputs(x, w_int8, w_scale)
    res = run_bass_kernel_spmd(nc, in_maps, core_ids=list(range(NCORES))).results
    inv = np.empty(NS, np.int64)
    inv[_PERM] = np.arange(NS)                        # orig_col -> new_col
    out = np.concatenate([res[c]["out"][:, inv] for c in range(NCORES)], axis=1)
    return out.reshape(orig_shape)
